# revision 22
# baseline (speedup 1.0000x reference)
"""Trainium2 Bass kernel for per-channel argmax box masking (local mask).

Semantics (matches the reference nn.Module):
  For each channel map m = x[b, c] of shape 56x56 (flattened 3136):
    idx = argmax(m); mi = idx // 56; mj = idx % 56
    h1 = clip(mi-3, 0, 55); h2 = clip(mi+3, 0, 55)   (exclusive upper)
    w1 = clip(mj-3, 0, 55); w2 = clip(mj+3, 0, 55)
    S = 1 everywhere, 0 inside box [h1,h2) x [w1,w2)
    lam = 3136 / (3136 - box_area)
    out = T[b,c] > 0 ? m * S * lam : m

Sharding strategy: channels with T == 0 are a pure identity (out == x), so
the host routes them straight into the output and only ships the ~50%
marked channels to the device, balanced across the 8 cores (padded to a
multiple of 128 per core). The device kernel computes the masked+scaled
values for its channels and returns them as fp16 (well inside the 2e-2
relative-error budget); unmarked channels stay bit-exact f32 on host.

Per 128-channel group on device:
  - hierarchical argmax: one full tensor_reduce(max) over [128,56,56]
    gives row maxima; a global reduce + max_index on the 56 row maxima
    gives the argmax ROW (mi) after only one full scan.
  - a 6-row window starting at rs=clip(mi-3,0,50) is gathered from x in
    DRAM by indirect DMA (the window always contains the argmax), and a
    max_index on those 336 elements recovers the argmax COLUMN (mj).
  - a tiny ALU chain derives the box, lam and scale factors.
  - the window correction  woutp = (rm x cm + sceff) * xw  uses a
    stride-0 broadcast outer product on GpSimd plus one fused
    scalar_tensor_tensor on DVE.
  - ACT scales each group IN PLACE, f32 -> f16, into the front half of
    the x tile through a bitcast view (the f16 write pointer always
    trails the f32 read pointer, so no copy and no second tile). The
    whole iteration then stores as ONE contiguous 12.5KB-per-partition
    HWDGE transfer. Dropping the separate output tile frees enough SBUF
    to hold a 5-deep x-tile ring, so the load queue never starves.
  - woutp and the window row starts return as small linear tensors and
    the HOST overlays the windows during unshard - no indirect scatter.

Schedule: the argmax stage (load, row reduce, gather issue) runs one
iteration AHEAD of the box/mask/scale stage, so each gather's ~3us round
trip hides under the next iteration's reduces.
"""

import numpy as np

import concourse.bass as bass
import concourse.bacc as bacc
import concourse.mybir as mybir
import concourse.tile as tile
from contextlib import ExitStack

F32 = mybir.dt.float32
F16 = mybir.dt.float16
I32 = mybir.dt.int32
U32 = mybir.dt.uint32

H = 56
HW = H * H          # 3136
WIN = 6 * H         # 336  (6-row window always contains the box rows)
N_CORES = 8
ALU = mybir.AluOpType
ACTF = mybir.ActivationFunctionType
NEG_INF = -3.4e38


def make_schedule(n_groups: int):
    """Iteration schedule: odd tail group first (its 1.6MB load lands
    ~5us before a pair's 3.2MB would, warming the pipeline), then pairs."""
    sched = [[k, k + 1] for k in range(0, n_groups - 1, 2)]
    if n_groups % 2 == 1:
        sched.insert(0, [n_groups - 1])
    return sched


def build_kernel(n_groups: int):
    """Build the per-core Bass program for n_groups 128-channel groups."""
    schedule = make_schedule(n_groups)
    n_iters = len(schedule)
    nch = n_groups * 128
    nc = bacc.Bacc("TRN2", target_bir_lowering=False, debug=False)

    x = nc.dram_tensor("x", [nch, HW], F32, kind="ExternalInput").ap()
    outs, wouts, rss = [], [], []
    for i, groups in enumerate(schedule):
        w = len(groups)
        outs.append(nc.dram_tensor(f"out{i}", [w * 128, HW], F16,
                                   kind="ExternalOutput").ap())
        wouts.append(nc.dram_tensor(f"wout{i}", [128, w * WIN], F16,
                                    kind="ExternalOutput").ap())
        rss.append(nc.dram_tensor(f"rs{i}", [128, w], F32,
                                  kind="ExternalOutput").ap())

    # channel-major views: [p, group, elem] and a row view for the gather
    x_g = x.rearrange("(n p) f -> p n f", p=128)
    x_rows = x.rearrange("a (r c) -> (a r) c", c=H)      # [nch*56, 56]
    # out{i} row (p*w + k): partition p's groups are adjacent, so the
    # whole iteration stores as one contiguous run per partition.
    out_p = [o.rearrange("(p n) f -> p (n f)", p=128) for o in outs]

    with ExitStack() as ctx:
        tc = ctx.enter_context(tile.TileContext(nc))
        cpool = ctx.enter_context(tc.tile_pool(name="consts", bufs=1))
        xpool = ctx.enter_context(tc.tile_pool(name="xtiles", bufs=5))
        tpool = ctx.enter_context(tc.tile_pool(name="xtail", bufs=1))
        wpool = ctx.enter_context(tc.tile_pool(name="wins", bufs=4))
        mpool = ctx.enter_context(
            tc.tile_pool(name="masks", bufs=2 * n_iters + 2))
        spool = ctx.enter_context(
            tc.tile_pool(name="scalars", bufs=2 * n_iters + 2))

        # constants generated on device: a DMA-loaded constant's completion
        # semaphore can get batched behind x-load completions on a shared
        # lane, stalling its first reader ~13us (seen on HW traces).
        crow_t = cpool.tile([128, 6], F32)
        ccol_t = cpool.tile([128, H], F32)
        pio56 = cpool.tile([128, 1], F32)
        crow_i = cpool.tile([128, 6], I32)
        ccol_i = cpool.tile([128, H], I32)
        pio_i = cpool.tile([128, 1], I32)
        nc.gpsimd.iota(crow_i[:], [[1, 6]], base=0, channel_multiplier=0)
        nc.gpsimd.iota(ccol_i[:], [[1, H]], base=0, channel_multiplier=0)
        nc.gpsimd.iota(pio_i[:], [[0, 1]], base=0, channel_multiplier=H)
        nc.gpsimd.tensor_copy(crow_t[:], crow_i[:])
        nc.gpsimd.tensor_copy(ccol_t[:], ccol_i[:])
        nc.gpsimd.tensor_copy(pio56[:], pio_i[:])

        # prewarm the ACT table (Copy) so real activations are fast
        warm = cpool.tile([128, 1], F32)
        nc.vector.memset(warm[:], 1.0)
        nc.scalar.activation(warm[:], warm[:], ACTF.Copy, bias=0.0, scale=1.0)

        ts = nc.vector.tensor_scalar
        tt = nc.vector.tensor_tensor

        state = {}

        def sc(tag, width, wt):
            return spool.tile([128, width], F32, tag=tag + wt, name=tag)

        def stageA(i):
            """Load, row argmax, gather issue for iteration i."""
            groups = schedule[i]
            w = len(groups)
            wt = "" if w == 2 else "T"
            pool = xpool if w == 2 else tpool
            xt = pool.tile([128, w * HW], F32, tag="xt" + wt)
            nc.sync.dma_start(
                xt[:].rearrange("p (n f) -> p n f", f=HW),
                x_g[:, groups[0] : groups[0] + w, :])

            xw = wpool.tile([128, w * WIN], F32, tag="xw" + wt)
            idxr = spool.tile([128, 8 * w], U32, tag="idxr" + wt)
            idxr3 = idxr[:].rearrange("p (g k) -> p g k", k=8)
            mib = sc("mib", w, wt)
            h1b = sc("h1b", w, wt)
            rsb = sc("rsb", w, wt)
            m8s = []

            for k in range(w):
                j = groups[k]
                xg3 = xt[:, k * HW : (k + 1) * HW].rearrange(
                    "p (r c) -> p r c", c=H)
                red56 = mpool.tile([128, H], F32, tag="red56")
                m8 = mpool.tile([128, 8], F32, tag="m8")
                nc.vector.tensor_reduce(red56[:], xg3, mybir.AxisListType.X,
                                        ALU.max)
                nc.vector.memset(m8[:], NEG_INF)
                nc.vector.tensor_reduce(m8[:, 0:1], red56[:],
                                        mybir.AxisListType.X, ALU.max)
                nc.vector.max_index(idxr3[:, k, :], m8[:], red56[:])
                m8s.append(m8)

                mi = mib[:, k : k + 1]
                h1 = h1b[:, k : k + 1]
                rs = rsb[:, k : k + 1]
                nc.vector.tensor_copy(mi, idxr3[:, k, 0:1])
                ts(h1, mi, -3.0, 0.0, ALU.add, ALU.max)
                ts(rs, h1, 50.0, None, ALU.min)
                gidxg = spool.tile([128, 1], I32, tag="gidxg")
                # global gather row = rs + j*128*H + p*H, no DRAM consts
                nc.vector.scalar_tensor_tensor(
                    gidxg[:], rs, float(j * 128 * H), pio56[:],
                    ALU.add, ALU.add)

                # window gather starts as soon as rs is known
                nc.gpsimd.indirect_dma_start(
                    out=xw[:, k * WIN : (k + 1) * WIN],
                    out_offset=None,
                    in_=x_rows,
                    in_offset=bass.IndirectOffsetOnAxis(ap=gidxg[:], axis=0),
                )

            # the host needs the window row starts to overlay woutp; store
            # from the Pool queue so the ACT queue head never waits on rsb
            nc.gpsimd.dma_start(rss[i], rsb[:])
            state[i] = (xt, xw, mib, h1b, rsb, m8s)

        def stageBCD(i):
            """Column argmax, box params, masks, scale, stores for iter i."""
            groups = schedule[i]
            w = len(groups)
            wt = "" if w == 2 else "T"
            xt, xw, mib, h1b, rsb, m8s = state.pop(i)
            woutp = wpool.tile([128, w * WIN], F16, tag="woutp" + wt)
            idxw = spool.tile([128, 8 * w], U32, tag="idxw" + wt)
            idxw3 = idxw[:].rearrange("p (g k) -> p g k", k=8)
            mjb = sc("mjb", w, wt)

            # ---- B/C interleaved: mjb-independent params run between the
            # column argmaxes
            nc.vector.max_index(idxw3[:, 0, :], m8s[0][:], xw[:, 0:WIN])
            dd = sc("dd", w, wt)
            tt(dd[:], mib[:], rsb[:], ALU.subtract)
            h2 = sc("h2", w, wt)
            ts(h2[:], mib[:], 3.0, 55.0, ALU.add, ALU.min)
            aa = sc("aa", w, wt)
            tt(aa[:], h1b[:], rsb[:], ALU.subtract)
            bb = sc("bb", w, wt)
            tt(bb[:], h2[:], rsb[:], ALU.subtract)
            bh = sc("bh", w, wt)
            tt(bh[:], h2[:], h1b[:], ALU.subtract)
            for k in range(1, w):
                nc.vector.max_index(idxw3[:, k, :], m8s[k][:],
                                    xw[:, k * WIN : (k + 1) * WIN])
            nc.vector.tensor_copy(mjb[:].unsqueeze(2), idxw3[:, :, 0:1])

            # mj = widx - 56*(mi - rs): no mod op needed, quotient is known
            nc.vector.scalar_tensor_tensor(
                mjb[:], dd[:], -56.0, mjb[:], ALU.mult, ALU.add)
            w1 = sc("w1", w, wt)
            ts(w1[:], mjb[:], -3.0, 0.0, ALU.add, ALU.max)
            w2 = sc("w2", w, wt)
            ts(w2[:], mjb[:], 3.0, 55.0, ALU.add, ALU.min)
            bw = sc("bw", w, wt)
            tt(bw[:], w2[:], w1[:], ALU.subtract)
            area = sc("area", w, wt)
            tt(area[:], bh[:], bw[:], ALU.mult)
            den = sc("den", w, wt)
            ts(den[:], area[:], -1.0, float(HW), ALU.mult, ALU.add)
            rcp = sc("rcp", w, wt)
            nc.vector.reciprocal(rcp[:], den[:])
            # every device channel is marked by construction: sceff = lam
            sceff = sc("sceff", w, wt)
            ts(sceff[:], rcp[:], float(HW), None, ALU.mult)
            bneg = sc("bneg", w, wt)
            ts(bneg[:], sceff[:], -1.0, None, ALU.mult)

            # ---- D: scale first (only needs sceff, so ACT overlaps the
            # window-mask work), then masks and the window correction
            xt16 = xt[:].bitcast(F16)        # [128, 2*w*HW] f16 view
            for k in range(w):
                # in-place f32 -> f16 scale: the write pointer (2B elems at
                # the tile front) always trails the read pointer (4B elems)
                nc.scalar.activation(xt16[:, k * HW : (k + 1) * HW],
                                     xt[:, k * HW : (k + 1) * HW],
                                     ACTF.Copy, bias=0.0,
                                     scale=sceff[:, k : k + 1])
            # one big store per iteration, right behind the scales
            nc.scalar.dma_start(out_p[i], xt16[:, 0 : w * HW])
            for k in range(w):
                sceff_g = sceff[:, k : k + 1]
                rm = mpool.tile([128, 6], F32, tag="rm")
                cm = mpool.tile([128, H], F32, tag="cm")
                ts(rm[:], crow_t[:], aa[:, k : k + 1], None, ALU.is_ge)
                nc.vector.scalar_tensor_tensor(
                    rm[:], crow_t[:], bb[:, k : k + 1], rm[:],
                    ALU.is_lt, ALU.mult)
                ts(rm[:], rm[:], bneg[:, k : k + 1], None, ALU.mult)
                ts(cm[:], ccol_t[:], w1[:, k : k + 1], None, ALU.is_ge)
                nc.vector.scalar_tensor_tensor(
                    cm[:], ccol_t[:], w2[:, k : k + 1], cm[:],
                    ALU.is_lt, ALU.mult)
                # mtmp[r, c] = rm[r] * cm[c]  (stride-0 broadcast outer)
                mtmp = mpool.tile([128, WIN], F32, tag="mtmp")
                mtmp3 = mtmp[:].rearrange("p (r c) -> p r c", c=H)
                nc.gpsimd.tensor_tensor(
                    mtmp3,
                    rm[:].unsqueeze(2).to_broadcast((128, 6, H)),
                    cm[:].unsqueeze(1).to_broadcast((128, 6, H)),
                    ALU.mult)
                # woutp = (mtmp + sceff) * xw fused on DVE (gpsimd
                # tensor_scalar with an AP scalar measures ~6us/op on HW)
                nc.vector.scalar_tensor_tensor(
                    woutp[:, k * WIN : (k + 1) * WIN],
                    mtmp[:], sceff_g, xw[:, k * WIN : (k + 1) * WIN],
                    ALU.add, ALU.mult)

            # windows go back as a small linear tensor
            nc.scalar.dma_start(wouts[i], woutp[:])

        stageA(0)
        if n_iters > 1:
            stageA(1)
        for i in range(n_iters):
            if i + 2 < n_iters:
                stageA(i + 2)
            stageBCD(i)

    nc.compile()
    return nc


def host_inputs(x_core: np.ndarray, n_groups: int):
    """Per-core input map. x_core [nch, 3136] f32 (all channels marked)."""
    nch = n_groups * 128
    assert x_core.shape == (nch, HW)
    return {"x": np.ascontiguousarray(x_core, dtype=np.float32)}


_CACHE = {}


def _get_nc(n_groups: int):
    if n_groups not in _CACHE:
        _CACHE[n_groups] = build_kernel(n_groups)
    return _CACHE[n_groups]


def kernel(x: np.ndarray, T: np.ndarray, _trace: bool = False):
    from concourse.bass_utils import run_bass_kernel_spmd

    B, C, Hh, Ww = x.shape
    assert (Hh, Ww) == (H, H)
    nch_total = B * C
    xf = np.ascontiguousarray(np.asarray(x, dtype=np.float32)).reshape(
        nch_total, HW)
    mb = np.asarray(T).reshape(-1) > 0
    midx = np.flatnonzero(mb)
    n_m = int(midx.size)

    # output starts as a copy of x; only marked channels get overwritten
    out = xf.copy()

    if n_m > 0:
        # pad the marked set to N_CORES * n_groups * 128 slots
        n_groups = -(-n_m // (N_CORES * 128))   # ceil
        per_core = n_groups * 128
        total = per_core * N_CORES
        slot = np.concatenate(
            [midx, np.full(total - n_m, midx[0], dtype=midx.dtype)])

        nc = _get_nc(n_groups)
        in_maps = [
            host_inputs(xf[slot[c * per_core : (c + 1) * per_core]], n_groups)
            for c in range(N_CORES)
        ]
        res = run_bass_kernel_spmd(nc, in_maps, list(range(N_CORES)),
                                   trace=_trace)

        schedule = make_schedule(n_groups)
        dev = np.empty((total, HW), dtype=np.float16)
        rs_all = np.empty(total, dtype=np.int64)
        wout_all = np.empty((total, WIN), dtype=np.float16)
        p = np.arange(128)
        for c in range(N_CORES):
            r = res.results[c]
            base = c * per_core
            for i, groups in enumerate(schedule):
                w = len(groups)
                o = r[f"out{i}"]                      # [w*128, HW]
                wo = r[f"wout{i}"]                    # [128, w*WIN]
                rsv = r[f"rs{i}"]                     # [128, w]
                for k, j in enumerate(groups):
                    sl = base + j * 128 + p
                    dev[sl] = o[p * w + k]
                    wout_all[sl] = wo[:, k * WIN : (k + 1) * WIN]
                    rs_all[sl] = rsv[:, k].astype(np.int64)
        # overlay the corrected 6-row windows at their per-channel rows
        col = rs_all[:, None] * H + np.arange(WIN)[None, :]
        np.put_along_axis(dev, col, wout_all, axis=1)
        out[midx] = dev[:n_m]   # fp16 -> f32 cast on assignment
    else:
        res = None

    out = out.reshape(B, C, Hh, Ww)
    if _trace:
        return out, res
    return out
